# revision 1
# baseline (speedup 1.0000x reference)
"""ContrastiveCenterLoss on 8 Trainium2 NeuronCores.

Math: with dist[b,c] = ||f_b - c_c||^2,
  intra = sum_b dist[b, label_b]          = sum_b ||f_b - c_{label_b}||^2
  total = sum_{b,c} dist[b,c]             = C*sum||f||^2 + B*sum||c||^2 - 2*(sum_b f_b)@(sum_c c_c)
  inter = total - intra
  loss  = (1/2/B) * intra / (inter + 1e-6) / 0.1

Sharding: feat/label batch-sharded (2048 rows/core); centers statistics
sharded over 512-row slices; the full centers table stays in HBM and is
row-gathered by label via indirect DMA. Host all-reduces the per-core
partial sums in float64 and applies the final scalar division.
"""

import numpy as np

B, C, D = 16384, 4096, 128
LAMBDA_C = 1.0
NCORES = 8
BS = B // NCORES          # 2048 feat rows per core
NPT = BS // 128           # 16 feat rows per partition
NCHUNK = 4                # feat processed in 4 chunks of 512 free-dim cols
CPC = NPT // NCHUNK       # 4 row-blocks per chunk
CS = C // NCORES          # 512 center rows per core (stats slice)
CSPT = CS // 128          # 4 center rows per partition

_cached = {}


def _build_nc(repeat=1, gather_mode="indirect"):
    import concourse.bass as bass
    import concourse.tile as tile
    from concourse import bacc, mybir

    f32 = mybir.dt.float32
    i32 = mybir.dt.int32

    nc = bacc.Bacc("TRN2", target_bir_lowering=False, debug=False,
                   num_devices=NCORES)

    feat = nc.dram_tensor("feat", [BS, D], f32, kind="ExternalInput")
    labt = nc.dram_tensor("labt", [128, NPT], i32, kind="ExternalInput")
    centers = nc.dram_tensor("centers", [C, D], f32, kind="ExternalInput")
    cslice = nc.dram_tensor("cslice", [CS, D], f32, kind="ExternalInput")

    o_fsq = nc.dram_tensor("o_fsq", [128, NCHUNK], f32, kind="ExternalOutput")
    o_intra = nc.dram_tensor("o_intra", [128, NCHUNK], f32, kind="ExternalOutput")
    o_csq = nc.dram_tensor("o_csq", [128, 1], f32, kind="ExternalOutput")
    o_vec = nc.dram_tensor("o_vec", [1, 1024], f32, kind="ExternalOutput")

    CW = CPC * D  # 512 free-dim columns per chunk

    with tile.TileContext(nc) as tc:
        with tc.tile_pool(name="const", bufs=1) as cpool, \
             tc.tile_pool(name="sbuf", bufs=2) as pool, \
             tc.tile_pool(name="scratch", bufs=2) as spool, \
             tc.tile_pool(name="psum", bufs=2, space="PSUM") as psum:

            ones = cpool.tile([128, 1], f32)
            nc.vector.memset(ones[:], 1.0)

            # partition p holds feat rows p*NPT .. p*NPT+NPT-1 (contiguous 8KB)
            fv = feat.ap().rearrange("(p n) d -> p n d", p=128)
            csv = cslice.ap().rearrange("(p n) d -> p n d", p=128)

            for _ in range(repeat):
                # indices first so gathers can start early
                lab = pool.tile([128, NPT], i32, tag="lab")
                nc.sync.dma_start(out=lab[:], in_=labt.ap())

                o_fsq_t = pool.tile([128, NCHUNK], f32, tag="o_fsq_t")
                o_intra_t = pool.tile([128, NCHUNK], f32, tag="o_intra_t")
                o_csq_t = pool.tile([128, 1], f32, tag="o_csq_t")
                vec_sb = pool.tile([1, 1024], f32, tag="vec_sb")

                ps_f = psum.tile([1, CW], f32, tag="ps_f")
                ps_c = psum.tile([1, CW], f32, tag="ps_c")

                # centers-slice statistics (independent of feat path)
                cs_t = pool.tile([128, CSPT * D], f32, tag="cs_t")
                nc.sync.dma_start(out=cs_t[:], in_=csv[:, :, :])
                cs_scr = pool.tile([128, CSPT * D], f32, tag="cs_scr")
                nc.scalar.activation(out=cs_scr[:], in_=cs_t[:],
                                     func=mybir.ActivationFunctionType.Square,
                                     accum_out=o_csq_t[:, 0:1])
                nc.tensor.matmul(out=ps_c[:], lhsT=ones[:], rhs=cs_t[:],
                                 start=True, stop=True)

                for k in range(NCHUNK):
                    f_c = spool.tile([128, CW], f32, tag="f_c")
                    nc.sync.dma_start(out=f_c[:],
                                      in_=fv[:, k * CPC:(k + 1) * CPC, :])
                    cg_c = spool.tile([128, CW], f32, tag="cg_c")
                    if gather_mode == "indirect":
                        for j in range(CPC):
                            nc.gpsimd.indirect_dma_start(
                                out=cg_c[:, j * D:(j + 1) * D],
                                out_offset=None,
                                in_=centers.ap(),
                                in_offset=bass.IndirectOffsetOnAxis(
                                    ap=lab[:, k * CPC + j:k * CPC + j + 1],
                                    axis=0),
                            )
                    else:  # "fake": plain DMA of same volume (timing expt)
                        cv = centers.ap().rearrange(
                            "(q p n) d -> q p n d", p=128, n=CPC)
                        nc.sync.dma_start(out=cg_c[:], in_=cv[k])
                    # sum of f^2 on ACT
                    f_scr = spool.tile([128, CW], f32, tag="f_scr")
                    nc.scalar.activation(
                        out=f_scr[:], in_=f_c[:],
                        func=mybir.ActivationFunctionType.Square,
                        accum_out=o_fsq_t[:, k:k + 1])
                    # column sums of f on PE (accumulated over chunks)
                    nc.tensor.matmul(out=ps_f[:], lhsT=ones[:], rhs=f_c[:],
                                     start=(k == 0), stop=(k == NCHUNK - 1))
                    # intra partial on DVE: d = f - cg; accum += d*d
                    d_c = spool.tile([128, CW], f32, tag="d_c")
                    nc.vector.tensor_sub(d_c[:], f_c[:], cg_c[:])
                    d_scr = spool.tile([128, CW], f32, tag="d_scr")
                    nc.vector.scalar_tensor_tensor(
                        out=d_scr[:], in0=d_c[:], scalar=1.0, in1=d_c[:],
                        op0=mybir.AluOpType.mult, op1=mybir.AluOpType.mult,
                        accum_out=o_intra_t[:, k:k + 1])

                nc.vector.tensor_copy(vec_sb[:, 0:CW], ps_f[:])
                nc.scalar.copy(vec_sb[:, CW:2 * CW], ps_c[:])

                nc.sync.dma_start(out=o_fsq.ap(), in_=o_fsq_t[:])
                nc.sync.dma_start(out=o_intra.ap(), in_=o_intra_t[:])
                nc.sync.dma_start(out=o_csq.ap(), in_=o_csq_t[:])
                nc.sync.dma_start(out=o_vec.ap(), in_=vec_sb[:])

    nc.compile()
    return nc


def _get_nc(repeat=1, gather_mode="indirect"):
    key = ("nc", repeat, gather_mode)
    if key not in _cached:
        _cached[key] = _build_nc(repeat, gather_mode)
    return _cached[key]


def _make_in_maps(feat, label, centers):
    feat = np.ascontiguousarray(np.asarray(feat, dtype=np.float32))
    centers = np.ascontiguousarray(np.asarray(centers, dtype=np.float32))
    lab = np.asarray(label).astype(np.int32)
    in_maps = []
    for k in range(NCORES):
        fs = feat[k * BS:(k + 1) * BS]
        ls = lab[k * BS:(k + 1) * BS].reshape(128, NPT)
        cs = centers[k * CS:(k + 1) * CS]
        in_maps.append({
            "feat": np.ascontiguousarray(fs),
            "labt": np.ascontiguousarray(ls),
            "centers": centers,
            "cslice": np.ascontiguousarray(cs),
        })
    return in_maps


def _combine(results):
    sum_fsq = 0.0
    intra = 0.0
    sum_csq = 0.0
    F = np.zeros(D, dtype=np.float64)
    Cv = np.zeros(D, dtype=np.float64)
    for r in results:
        sum_fsq += r["o_fsq"].astype(np.float64).sum()
        intra += r["o_intra"].astype(np.float64).sum()
        sum_csq += r["o_csq"].astype(np.float64).sum()
        v = r["o_vec"][0].astype(np.float64)
        F += v[:512].reshape(4, 128).sum(axis=0)
        Cv += v[512:].reshape(4, 128).sum(axis=0)
    total = C * sum_fsq + B * sum_csq - 2.0 * float(F @ Cv)
    inter = total - intra
    loss = (LAMBDA_C / 2.0 / B) * intra / (inter + 1e-6) / 0.1
    return np.float32(loss)


def kernel(feat, label, centers):
    from concourse.bass_utils import run_bass_kernel_spmd

    nc = _get_nc()
    in_maps = _make_in_maps(feat, label, centers)
    res = run_bass_kernel_spmd(nc, in_maps, list(range(NCORES)))
    return _combine(res.results)



# revision 2
# speedup vs baseline: 2.0519x; 2.0519x over previous
"""ContrastiveCenterLoss on 8 Trainium2 NeuronCores — v2.

Math: with dist[b,c] = ||f_b - c_c||^2,
  intra = sum_b dist[b, label_b] = sum f^2 + sum cg^2 - 2*sum f.cg
          (cg = centers gathered by label)
  total = C*sum||f||^2 + B*sum||c||^2 - 2*(sum_b f_b)@(sum_c c_c)
  inter = total - intra
  loss  = (1/2/B) * intra / (inter + 1e-6) / 0.1

v2 changes vs baseline:
  - one dma_gather per chunk (SWDGE Q7 gather, 994ns fixed cost each)
    instead of 16 indirect_dma_start calls (16.6us of Pool time).
  - feat + centers-slice shipped as one bf16 blob (halved bytes, one
    HWDGE transaction); gather stays 512B/row-equivalent cost.
  - column sums via per-block matmul with ones as the *moving* operand
    (out [128,1] psum accumulated across blocks) -> all outputs are
    [128, k]; a single small output DMA.
  - gather split (1024, 896, 128) so the tail chunk's compute is tiny.
Host all-reduces the per-core partial sums in float64 and applies the
final scalar division.
"""

import numpy as np
import ml_dtypes

B, C, D = 16384, 4096, 128
LAMBDA_C = 1.0
NCORES = 8
BS = B // NCORES          # 2048 feat rows per core
NB = BS // 128            # 16 feat row-blocks of 128
CSL = C // NCORES         # 512 center rows per core (stats slice)
CB = CSL // 128           # 4 cslice row-blocks
GCH = (896, 640, 512)     # gather chunk sizes (sum = BS)
NOCOL = 12                # output columns

_cached = {}


def _build_nc():
    import concourse.bass as bass
    import concourse.tile as tile
    from concourse import bacc, mybir

    f32 = mybir.dt.float32
    bf16 = mybir.dt.bfloat16
    i16 = mybir.dt.int16
    mult = mybir.AluOpType.mult

    nc = bacc.Bacc("TRN2", target_bir_lowering=False, debug=False,
                   num_devices=NCORES, dynamic_dma_scratch_size=65536)

    idxt = nc.dram_tensor("idxt", [128, BS // 16], i16, kind="ExternalInput")
    blob = nc.dram_tensor("blob", [128, (NB + CB) * D], bf16,
                          kind="ExternalInput")
    censb = nc.dram_tensor("censb", [C, D], bf16, kind="ExternalInput")
    # cols: 0 fsq | 1 csq | 2 F | 3 Cv | 4.. cross spans | .. cgsq spans
    o_all = nc.dram_tensor("o_all", [128, NOCOL], f32, kind="ExternalOutput")

    FW = NB * D               # 2048 feat free cols
    CW = CB * D               # 512 cslice free cols

    with tile.TileContext(nc) as tc:
        with tc.tile_pool(name="const", bufs=1) as cpool, \
             tc.tile_pool(name="sbuf", bufs=1) as pool, \
             tc.tile_pool(name="psum", bufs=1, space="PSUM") as psum:

            ones = cpool.tile([128, 1], bf16, tag="ones")
            nc.vector.memset(ones[:], 1.0)

            # index tile first so the gather chain starts ASAP (HWDGE)
            idx_t = pool.tile([128, BS // 16], i16, tag="idx")
            nc.sync.dma_start(out=idx_t[:], in_=idxt.ap())

            # feat + cslice blob via SWDGE so its desc-gen overlaps the
            # idx HWDGE transaction and the transfer starts early
            bl_t = pool.tile([128, (NB + CB) * D], bf16, tag="blob")
            nc.gpsimd.dma_start(out=bl_t[:], in_=blob.ap())
            f_v = bl_t[:, 0:FW]
            cs_v = bl_t[:, FW:FW + CW]

            cg_t = pool.tile([128, FW], bf16, tag="cg")
            cg3 = cg_t[:].rearrange("p (n d) -> p n d", d=D)

            o_t = pool.tile([128, NOCOL], f32, tag="o")

            # gathers: centers rows by label, chunked
            s = 0
            for gi, n in enumerate(GCH):
                nc.gpsimd.dma_gather(
                    cg3[:, s // 128:(s + n) // 128, :],
                    censb.ap(),
                    idx_t[:, s // 16:(s + n) // 16],
                    n, n, D,
                )
                s += n

            # ---- per-engine streams. The Tile list scheduler can reorder
            # within an engine, so same-engine order is pinned with 1-column
            # write overlaps (WAW deps) into a shared scratch per engine. ----

            # Output columns:
            # 0 f2a(DVE) 1 f2b(ACT) 2 csq 3 F 4 Cv
            # 5,6,7 cross1..3 (DVE)  8 cg1^2 9 cg2^2 10 cg3a^2 (ACT)
            # 11 cg3b^2 (DVE)
            SQ = mybir.ActivationFunctionType.Square
            X3 = 432          # wave-3 square rows on ACT; rest on DVE

            # DVE stream: f2a, psum copies, cross1..3, cg3b^2
            s_d = pool.tile([128, 3072 + 96], bf16, tag="s_d")
            nc.vector.scalar_tensor_tensor(
                out=s_d[:, 0:1024], in0=f_v[:, 0:1024], scalar=1.0,
                in1=f_v[:, 0:1024], op0=mult, op1=mult,
                accum_out=o_t[:, 0:1])

            # ACT stream: cs^2, f2b, cg1^2, cg2^2, cg3a^2
            s_a = pool.tile([128, 3520], bf16, tag="s_a")
            nc.scalar.activation(
                out=s_a[:, 0:CW], in_=cs_v, func=SQ, accum_out=o_t[:, 2:3])
            nc.scalar.activation(
                out=s_a[:, CW - 1:CW - 1 + 1024], in_=f_v[:, 1024:2048],
                func=SQ, accum_out=o_t[:, 1:2])

            # column sums: per-block matmul, data stationary, ones moving
            # -> [128,1] psum accumulated across blocks
            psF = psum.tile([128, 1], f32, tag="psF")
            f3 = bl_t[:].rearrange("p (n d) -> p n d", d=D)
            for n in range(NB):
                nc.tensor.matmul(out=psF[:], lhsT=f3[:, n, :], rhs=ones[:],
                                 start=(n == 0), stop=(n == NB - 1))
            psC = psum.tile([128, 1], f32, tag="psC")
            for n in range(CB):
                nc.tensor.matmul(out=psC[:], lhsT=f3[:, NB + n, :],
                                 rhs=ones[:],
                                 start=(n == 0), stop=(n == CB - 1))
            # psum -> o_t copies on DVE (idle pre-gather window), pinned
            # after f2a via s_d overlap of a dummy... copies write o_t only;
            # they are tiny and ready early, emission order suffices.
            nc.vector.tensor_copy(o_t[:, 3:4], psF[:])
            nc.vector.tensor_copy(o_t[:, 4:5], psC[:])

            # gather-gated: cross_i on DVE (pinned chain), cg_i^2 on ACT
            b1, b2 = GCH[0], GCH[0] + GCH[1]
            nc.vector.scalar_tensor_tensor(
                out=s_d[:, 1023:1023 + b1], in0=f_v[:, 0:b1], scalar=1.0,
                in1=cg_t[:, 0:b1], op0=mult, op1=mult,
                accum_out=o_t[:, 5:6])
            nc.vector.scalar_tensor_tensor(
                out=s_d[:, 1918:1918 + GCH[1]], in0=f_v[:, b1:b2],
                scalar=1.0, in1=cg_t[:, b1:b2], op0=mult, op1=mult,
                accum_out=o_t[:, 6:7])
            nc.vector.scalar_tensor_tensor(
                out=s_d[:, 2557:2557 + GCH[2]], in0=f_v[:, b2:FW],
                scalar=1.0, in1=cg_t[:, b2:FW], op0=mult, op1=mult,
                accum_out=o_t[:, 7:8])
            # wave-3 square tail piece on DVE after cross3
            nc.vector.scalar_tensor_tensor(
                out=s_d[:, 3068:3068 + (FW - b2 - X3)],
                in0=cg_t[:, b2 + X3:FW], scalar=1.0,
                in1=cg_t[:, b2 + X3:FW], op0=mult, op1=mult,
                accum_out=o_t[:, 11:12])

            # ACT: cg1^2, cg2^2, cg3a^2 pinned chain after f2b
            nc.scalar.activation(
                out=s_a[:, 1534:1534 + b1], in_=cg_t[:, 0:b1], func=SQ,
                accum_out=o_t[:, 8:9])
            nc.scalar.activation(
                out=s_a[:, 1534 + b1 - 1:1534 + b1 - 1 + GCH[1]],
                in_=cg_t[:, b1:b2], func=SQ, accum_out=o_t[:, 9:10])
            nc.scalar.activation(
                out=s_a[:, 1532 + b1 + GCH[1]:1532 + b1 + GCH[1] + X3],
                in_=cg_t[:, b2:b2 + X3],
                func=SQ, accum_out=o_t[:, 10:11])

            nc.sync.dma_start(out=o_all.ap(), in_=o_t[:])

    nc.compile()
    return nc


def _get_nc():
    if "nc" not in _cached:
        _cached["nc"] = _build_nc()
    return _cached["nc"]


def _make_in_maps(feat, label, centers):
    feat = np.asarray(feat, dtype=np.float32)
    centers = np.asarray(centers, dtype=np.float32)
    lab = np.asarray(label).astype(np.int16)

    bf = ml_dtypes.bfloat16
    censb = np.ascontiguousarray(centers.astype(bf))

    in_maps = []
    for k in range(NCORES):
        fs = feat[k * BS:(k + 1) * BS].astype(bf)
        # row i -> partition i%128, block i//128
        fs = fs.reshape(NB, 128, D).transpose(1, 0, 2)
        cs = centers[k * CSL:(k + 1) * CSL].astype(bf)
        cs = cs.reshape(CB, 128, D).transpose(1, 0, 2)
        blob = np.ascontiguousarray(
            np.concatenate([fs.reshape(128, NB * D),
                            cs.reshape(128, CB * D)], axis=1))
        # gather idx layout: position i read from idx[i%16, i//16];
        # partitions 16..127 replicate (must hold valid indices)
        ls = lab[k * BS:(k + 1) * BS]
        m16 = np.ascontiguousarray(ls.reshape(BS // 16, 16).T)  # [16, BS/16]
        idx = np.ascontiguousarray(np.tile(m16, (8, 1)))        # [128, BS/16]
        in_maps.append({
            "idxt": idx,
            "blob": blob,
            "censb": censb,
        })
    return in_maps


def _combine(results):
    fsq = 0.0
    cross = 0.0
    cgsq = 0.0
    csq = 0.0
    F = np.zeros(D, dtype=np.float64)
    Cv = np.zeros(D, dtype=np.float64)
    for r in results:
        a = r["o_all"].astype(np.float64)
        fsq += a[:, 0:2].sum()
        csq += a[:, 2].sum()
        F += a[:, 3]
        Cv += a[:, 4]
        cross += a[:, 5:8].sum()
        cgsq += a[:, 8:12].sum()
    intra = fsq + cgsq - 2.0 * cross
    total = C * fsq + B * csq - 2.0 * float(F @ Cv)
    inter = total - intra
    loss = (LAMBDA_C / 2.0 / B) * intra / (inter + 1e-6) / 0.1
    return np.float32(loss)


def kernel(feat, label, centers):
    from concourse.bass_utils import run_bass_kernel_spmd

    nc = _get_nc()
    in_maps = _make_in_maps(feat, label, centers)
    res = run_bass_kernel_spmd(nc, in_maps, list(range(NCORES)))
    return _combine(res.results)
